# revision 7
# baseline (speedup 1.0000x reference)
"""Multi-head graph attention (rank-2 LeakyReLU-softmax) Trainium2 kernel.

Reference computation (per batch b, head h):
    V = X @ vW + vb                       (N, F)
    q = V @ qW[:,h] + qb[h]               (N,)   per-node scalar
    k = V @ kW[:,h] + kb[h]               (N,)
    A_ij = softmax_j( LeakyReLU(q_i * k_j) )
    out[b,i,h,:] = sum_j A_ij V_j

Key identity used here: with P = max(q,0), M = min(q,0),
alpha = LeakyReLU(k) = max(k, 0.01k), beta = min(k, 0.01k),
    LeakyReLU(q_i * k_j) == alpha_j * P_i + beta_j * M_i      (exactly)
since for each i exactly one of P_i / M_i is nonzero.  So the N x N logit
matrix is a rank-2 outer product, built on the TensorEngine as a K=2
matmul (fp32r), exponentiated on the ScalarEngine straight out of PSUM,
and contracted against [V | 1] without the N x N matrix ever leaving the
chip.  The trailing all-ones column of Vp1 yields the softmax denominator
as row 64 of the same accumulation.

Sharding: core c -> batch b = c//2, heads h0 = 4*(c%2) .. h0+3.
"""

import numpy as np

import concourse.bacc as bacc
import concourse.tile as tile
import concourse.mybir as mybir
from concourse.bass_utils import run_bass_kernel_spmd

B, N, IN, F, H = 4, 2048, 256, 64, 8
NH = H // 2          # heads per core
NT = N // 128        # 16 i-tiles / j-chunks
F32 = mybir.dt.float32
F32R = mybir.dt.float32r
AF = mybir.ActivationFunctionType
ALU = mybir.AluOpType

N_CORES = 8
_CACHE = {}


def build_nc():
    nc = bacc.Bacc("TRN2", target_bir_lowering=False, debug=False,
                   num_devices=N_CORES)
    X_d = nc.dram_tensor("X", [N, IN], F32, kind="ExternalInput")
    vW_d = nc.dram_tensor("vW", [IN, F], F32, kind="ExternalInput")
    vb_d = nc.dram_tensor("vb", [F], F32, kind="ExternalInput")
    qw_d = nc.dram_tensor("qw", [F, NH], F32, kind="ExternalInput")
    kw_d = nc.dram_tensor("kw", [F, NH], F32, kind="ExternalInput")
    qb_d = nc.dram_tensor("qb", [NH], F32, kind="ExternalInput")
    kb_d = nc.dram_tensor("kb", [NH], F32, kind="ExternalInput")
    id_d = nc.dram_tensor("ident", [128, 128], F32, kind="ExternalInput")
    out_d = nc.dram_tensor("out", [N, NH * F], F32, kind="ExternalOutput")

    with tile.TileContext(nc) as tc:
        with tc.tile_pool(name="persist", bufs=1) as pp:
            ident = pp.tile([128, 128], F32)
            nc.sync.dma_start(ident[:], id_d[:])
            vt_sb = pp.tile([F, N], F32)          # V^T, bias folded
            qt = pp.tile([NH, N], F32)
            kt = pp.tile([NH, N], F32)
            ab_hs = [pp.tile([2, N], F32R, name=f"abh{h}", tag=f"ab{h}") for h in range(NH)]
            pm_hs = [pp.tile([2, N], F32R, name=f"pmh{h}", tag=f"pm{h}") for h in range(NH)]
            vp1 = pp.tile([128, NT * (F + 1)], F32R)   # [V | 1] per j-tile

            # ---------- preamble: X^T, V^T, q/k ----------
            with tc.tile_pool(name="pre_sb", bufs=1) as sp:
                xsb = sp.tile([128, NT * IN], F32)
                nc.sync.dma_start(
                    xsb[:].rearrange("p (t c) -> p t c", t=NT),
                    X_d[:].rearrange("(t p) c -> p t c", p=128))
                vwsb = sp.tile([128, 128], F32)
                nc.sync.dma_start(
                    vwsb[:].rearrange("p (t f) -> p t f", t=2),
                    vW_d[:].rearrange("(t p) f -> p t f", p=128))
                vb_t = sp.tile([F, 1], F32)
                nc.sync.dma_start(vb_t[:], vb_d[:].unsqueeze(1))
                qw_t = sp.tile([F, NH], F32)
                nc.sync.dma_start(qw_t[:], qw_d[:])
                kw_t = sp.tile([F, NH], F32)
                nc.sync.dma_start(kw_t[:], kw_d[:])
                qb_t = sp.tile([NH, 1], F32)
                nc.sync.dma_start(qb_t[:], qb_d[:].unsqueeze(1))
                kb_t = sp.tile([NH, 1], F32)
                nc.sync.dma_start(kb_t[:], kb_d[:].unsqueeze(1))

                xt = sp.tile([128, 2 * N], F32)   # X^T: chunk cc at cc*N
                with tc.tile_pool(name="pre_ps", bufs=2, space="PSUM") as xp:
                    for t in range(NT):
                        for cc in range(2):
                            tp = xp.tile([128, 128], F32)
                            nc.tensor.transpose(
                                tp[:], xsb[:, t * IN + cc * 128:
                                           t * IN + cc * 128 + 128], ident[:])
                            nc.vector.tensor_copy(
                                xt[:, cc * N + t * 128: cc * N + t * 128 + 128],
                                tp[:])

                with tc.tile_pool(name="vt_ps", bufs=1, space="PSUM") as vpp:
                    vt_ps = vpp.tile([F, N], F32)
                    for nb in range(4):
                        for cc in range(2):
                            nc.tensor.matmul(
                                vt_ps[:, nb * 512: nb * 512 + 512],
                                vwsb[:, cc * F: cc * F + F],
                                xt[:, cc * N + nb * 512: cc * N + nb * 512 + 512],
                                start=(cc == 0), stop=(cc == 1))
                    nc.vector.tensor_scalar_add(vt_sb[:], vt_ps[:], vb_t[:])

                with tc.tile_pool(name="qk_ps", bufs=1, space="PSUM") as qpp:
                    qt_ps = qpp.tile([NH, N], F32)
                    kt_ps = qpp.tile([NH, N], F32)
                    for nb in range(4):
                        nc.tensor.matmul(
                            qt_ps[:, nb * 512: nb * 512 + 512], qw_t[:],
                            vt_sb[:, nb * 512: nb * 512 + 512],
                            start=True, stop=True)
                        nc.tensor.matmul(
                            kt_ps[:, nb * 512: nb * 512 + 512], kw_t[:],
                            vt_sb[:, nb * 512: nb * 512 + 512],
                            start=True, stop=True)
                    nc.vector.tensor_scalar_add(qt[:], qt_ps[:], qb_t[:])
                    nc.vector.tensor_scalar_add(kt[:], kt_ps[:], kb_t[:])

            # ---------- per-head vectors (fp32r) ----------
            with tc.tile_pool(name="vec_sb", bufs=1) as vs:
                a4 = vs.tile([NH, N], F32R)
                b4 = vs.tile([NH, N], F32R)
                p4 = vs.tile([NH, N], F32R)
                m4 = vs.tile([NH, N], F32R)
                nc.vector.scalar_tensor_tensor(a4[:], kt[:], 0.01, kt[:],
                                               ALU.mult, ALU.max)
                nc.vector.scalar_tensor_tensor(b4[:], kt[:], 0.01, kt[:],
                                               ALU.mult, ALU.min)
                nc.vector.tensor_scalar_max(p4[:], qt[:], 0.0)
                nc.vector.tensor_scalar_min(m4[:], qt[:], 0.0)
                for h in range(NH):
                    nc.sync.dma_start(ab_hs[h][0:1, :], a4[h:h + 1, :])
                    nc.sync.dma_start(ab_hs[h][1:2, :], b4[h:h + 1, :])
                    nc.sync.dma_start(pm_hs[h][0:1, :], p4[h:h + 1, :])
                    nc.sync.dma_start(pm_hs[h][1:2, :], m4[h:h + 1, :])

            # ---------- Vp1 = [V | 1] per j-tile ----------
            nc.vector.memset(vp1[:].bitcast(F32), 1.0)
            with tc.tile_pool(name="v_ps", bufs=2, space="PSUM") as vp:
                for t in range(NT):
                    v_ps = vp.tile([128, F], F32)
                    nc.tensor.transpose(
                        v_ps[:], vt_sb[:, t * 128: t * 128 + 128],
                        ident[0:F, 0:F])
                    nc.vector.tensor_copy(
                        vp1[:, t * (F + 1): t * (F + 1) + F], v_ps[:])

            # ---------- main loop ----------
            with tc.tile_pool(name="lt_ps", bufs=2, space="PSUM") as ltp, \
                 tc.tile_pool(name="acc_ps", bufs=1, space="PSUM") as accp, \
                 tc.tile_pool(name="ht_ps", bufs=2, space="PSUM") as htp, \
                 tc.tile_pool(name="et_sb", bufs=3) as etp, \
                 tc.tile_pool(name="post_sb", bufs=2) as postp:
                for h in range(NH):
                    ab_h = ab_hs[h][:]
                    pm_h = pm_hs[h][:]
                    for ib in range(2):
                        acc = accp.tile([F + 1, 1024], F32, tag="acc")
                        for jc in range(NT):
                            lt = ltp.tile([128, 1024], F32, tag="lt")
                            for hf in range(2):
                                nc.tensor.matmul(
                                    lt[:, hf * 512: hf * 512 + 512],
                                    ab_h[:, jc * 128: jc * 128 + 128],
                                    pm_h[:, ib * 1024 + hf * 512:
                                         ib * 1024 + hf * 512 + 512],
                                    start=True, stop=True)
                            et = etp.tile([128, 1024], F32R, tag="et")
                            nc.scalar.activation(et[:], lt[:], AF.Exp)
                            for hf in range(2):
                                nc.tensor.matmul(
                                    acc[:, hf * 512: hf * 512 + 512],
                                    vp1[:, jc * (F + 1): (jc + 1) * (F + 1)],
                                    et[:, hf * 512: hf * 512 + 512],
                                    start=(jc == 0), stop=(jc == NT - 1))
                        hsb = postp.tile([F + 1, 1024], F32, tag="hsb")
                        nc.vector.tensor_copy(hsb[:], acc[:])
                        for t8 in range(8):
                            ht = htp.tile([128, F + 1], F32, tag="ht")
                            nc.tensor.transpose(
                                ht[:], hsb[:, t8 * 128: t8 * 128 + 128],
                                ident[0:F + 1, 0:F + 1])
                            rcp = postp.tile([128, 1], F32, tag="rcp")
                            nc.vector.reciprocal(rcp[:], ht[:, F:F + 1])
                            ob = postp.tile([128, F], F32, tag="ob")
                            nc.vector.tensor_scalar_mul(ob[:], ht[:, 0:F], rcp[:])
                            r0 = ib * 1024 + t8 * 128
                            nc.sync.dma_start(
                                out_d[r0:r0 + 128, h * F: h * F + F], ob[:])
    nc.compile()
    return nc


def _get_nc():
    if "nc" not in _CACHE:
        _CACHE["nc"] = build_nc()
    return _CACHE["nc"]


def make_in_maps(X, vW, vb, qW, qb, kW, kb):
    ident = np.eye(128, dtype=np.float32)
    in_maps = []
    for c in range(N_CORES):
        b, h0 = c // 2, NH * (c % 2)
        in_maps.append({
            "X": np.ascontiguousarray(X[b]),
            "vW": np.ascontiguousarray(vW),
            "vb": np.ascontiguousarray(vb),
            "qw": np.ascontiguousarray(qW[:, h0:h0 + NH]),
            "kw": np.ascontiguousarray(kW[:, h0:h0 + NH]),
            "qb": np.ascontiguousarray(qb[h0:h0 + NH]),
            "kb": np.ascontiguousarray(kb[h0:h0 + NH]),
            "ident": ident,
        })
    return in_maps


def assemble(results):
    full = np.empty((B, N, H * F), dtype=np.float32)
    for c in range(N_CORES):
        b, h0 = c // 2, NH * (c % 2)
        full[b][:, h0 * F:(h0 + NH) * F] = results[c]["out"]
    return full


def kernel(X, vW, vb, qW, qb, kW, kb):
    X, vW, vb = np.asarray(X), np.asarray(vW), np.asarray(vb)
    qW, qb, kW, kb = np.asarray(qW), np.asarray(qb), np.asarray(kW), np.asarray(kb)
    nc = _get_nc()
    res = run_bass_kernel_spmd(nc, make_in_maps(X, vW, vb, qW, qb, kW, kb),
                               list(range(N_CORES)))
    return assemble(res.results)
